# revision 1
# baseline (speedup 1.0000x reference)
"""Trainium2 Bass kernel for nn_Aggregate (gate-softmax graph pooling).

Computes, for each graph b:
    gate[b,n] = x[b,n,:] @ W1 + b1
    attn      = softmax(gate[b,:])
    y[b,:]    = sum_n attn[b,n] * x[b,n,:]

Strategy (memory-bound; roofline = one HBM read of x):
  - Data-parallel over the 32 graphs: 4 graphs per NeuronCore, 8 cores.
  - Single pass over x. gate values are ~N(0,1) so exp() without the
    max-shift is safe in fp32; softmax = (sum e^g x) / (sum e^g) needs
    no running-max correction, so every x element is read from HBM once.
  - Per 1 MiB slab (1024 nodes as [128 partitions x 8 nodes x 256 feat]):
      DVE    : g1 = x * W1rep flat multiply for 6 node-groups, grouped
               1x reduce for 5 groups
      GpSimd : the other 2 node-groups of the multiply
      ACT    : reduce of remaining 3 groups via activation(Copy, accum_out),
               then exp(gates + b1) whose accum_out gives sum(e^g)/partition
      PE     : per group, matmul(psum[1,256] += e^g col [128,1].T @ x
               [128,256]) accumulated over the graph's 64 groups; this fp32
               MM stream (2 HW passes each) is the ~110us/core bottleneck
  - Denominator finishes on host: sum of the per-partition exp-sums.
  - First slab of each graph is split 2+2+4 groups (short gate chains keep
    PE fed at graph starts); W1/b1 broadcasts ride ahead on the sync queue;
    a dummy exp preloads ACT's table set during the preamble.
"""

import sys
import types

import numpy as np

import concourse.bass as bass
import concourse.tile as tile
from concourse import mybir
from concourse.bass_utils import run_bass_kernel_spmd

# bass_utils' axon trace path does `from antenv.axon_hooks import ...`, which
# this image doesn't ship; stub it so BASS_TRACE=1 degrades to a warning
# instead of an ImportError. (Our own profiling wires a real hook in test.py.)
if "antenv.axon_hooks" not in sys.modules:
    try:
        import antenv  # noqa: F401
        import antenv.axon_hooks  # noqa: F401
    except ImportError:
        _m = types.ModuleType("antenv.axon_hooks")
        _m._hook = None
        _m.set_axon_ntff_profile_hook = lambda h: setattr(_m, "_hook", h)
        _m.get_axon_ntff_profile_hook = lambda: _m._hook
        sys.modules["antenv.axon_hooks"] = _m

BZ, N, F = 32, 8192, 256
NCORES = 8
BZL = BZ // NCORES  # graphs per core
P = 128             # SBUF partitions
JJ = 8              # nodes per partition per slab
SLAB = P * JJ       # 1024 nodes per slab
FP32 = mybir.dt.float32


def split_multiwait(nc) -> int:
    """Walrus in this image only encodes one sync-wait per instruction for
    ctrl-class ops; hoist extra waits onto single-wait NoOps just before."""
    n_fixed = 0
    for fn in nc.m.functions:
        for blk in fn.blocks:
            new_list = []
            for inst in blk.instructions:
                si = inst.sync_info
                waits = list(si.on_wait) if si is not None else []
                if len(waits) > 1:
                    for k, w in enumerate(waits):
                        new_list.append(
                            mybir.InstNoOp(
                                name=f"{inst.name}-wsplit{k}",
                                engine=inst.engine,
                                sync_info=mybir.SyncInfo(on_wait=[w], on_update=[]),
                                bass_nofuse=True,
                            )
                        )
                    inst.sync_info = mybir.SyncInfo(
                        on_wait=[], on_update=list(si.on_update)
                    )
                    n_fixed += 1
                new_list.append(inst)
            blk.instructions = new_list
    return n_fixed


def build(n_nodes: int = N, bzl: int = BZL, fixup: bool = True) -> bass.Bass:
    nslab = n_nodes // SLAB
    assert nslab * SLAB == n_nodes

    nc = bass.Bass("TRN2", target_bir_lowering=False, debug=False)
    x_d = nc.dram_tensor("x", [bzl, n_nodes, F], FP32, kind="ExternalInput").ap()
    w1_d = nc.dram_tensor("W1", [F, 1], FP32, kind="ExternalInput").ap()
    b1_d = nc.dram_tensor("b1", [1], FP32, kind="ExternalInput").ap()
    y_d = nc.dram_tensor("y_unnorm", [bzl, 2, 2 * F], FP32, kind="ExternalOutput").ap()
    ws_d = nc.dram_tensor("wsum", [bzl, P, nslab + 2], FP32, kind="ExternalOutput").ap()

    with tile.TileContext(nc) as tc:
        with (
            tc.tile_pool(name="singles", bufs=1) as singles,
            tc.tile_pool(name="xp", bufs=6) as xp,
            tc.tile_pool(name="g1p", bufs=4) as g1p,
            tc.tile_pool(name="small", bufs=4) as small,
            tc.tile_pool(name="scr", bufs=2) as scrp,
            tc.tile_pool(name="wsump", bufs=3) as wsump,
            tc.tile_pool(name="outp", bufs=2) as outp,
            tc.tile_pool(name="psum", bufs=2, space="PSUM") as psump,
        ):
            # b1 scalar broadcast to [128,1] (the exp bias) — first in the
            # sync queue so the first chunk's exp is never blocked on it.
            b1b = singles.tile([P, 1], FP32)
            nc.sync.dma_start(
                out=b1b,
                in_=bass.AP(tensor=b1_d.tensor, offset=b1_d.offset, ap=[[0, P], [1, 1]]),
            )
            # W1 (256 contiguous fp32) broadcast to [128, 256].
            w1rep = singles.tile([P, F], FP32)
            nc.sync.dma_start(
                out=w1rep,
                in_=bass.AP(tensor=w1_d.tensor, offset=w1_d.offset, ap=[[0, P], [1, F]]),
            )
            w1r_ap = w1rep[:, :]
            w1rep_bc = bass.AP(
                tensor=w1r_ap.tensor,
                offset=w1r_ap.offset,
                ap=[list(w1r_ap.ap[0]), [0, JJ], list(w1r_ap.ap[1])],
            )
            # Materialized [128, 8, 256] copy: unit-stride in1 keeps the big
            # DVE multiplies on the flat-2D fast path (stride-0 views cost
            # ~+400ns/op there). Filled by a SBUF->SBUF broadcast DMA (zero
            # engine time; engine copies port-block the DVE) issued after the
            # priming chunks' loads — see the chunk loop.
            w1rep8 = singles.tile([P, JJ, F], FP32)
            # Dummy exp so ACT's table set loads during the preamble instead
            # of on the first real exp.
            warm = singles.tile([P, 1], FP32)
            nc.scalar.activation(
                out=warm, in_=b1b, func=mybir.ActivationFunctionType.Exp,
                bias=0.0, scale=1.0,
            )
            # Flatten all (graph, chunk) work items so each graph's epilogue
            # can be emitted (priority-wise) after the next graph's pipeline
            # has started — keeps PE fed across graph boundaries.
            items = []
            for b in range(bzl):
                chunks = []
                for s in range(nslab):
                    if s == 0:
                        # split each graph's first slab: short gate chains
                        # keep PE fed across graph boundaries
                        chunks += [(0, 2), (P * 2, 2), (P * 4, JJ - 4)]
                    else:
                        chunks.append((s * SLAB, JJ))
                for ci, (n0, jj) in enumerate(chunks):
                    items.append((b, ci, len(chunks), n0, jj))

            wsums = {}
            psums = {}

            def epilogue(b):
                yrow = outp.tile([2, 2 * F], FP32)
                nc.scalar.copy(yrow, psums[b])
                nc.sync.dma_start(out=y_d[b], in_=yrow)
                nc.sync.dma_start(out=ws_d[b], in_=wsums[b])

            for (b, ci, nchunks, n0, jj) in items:
                if ci == 0:
                    wsums[b] = wsump.tile([P, nslab + 2], FP32, tag="wsum_cols", name=f"wsum_{b}")
                    nc.vector.memset(wsums[b][:, nslab : nslab + 2], 0.0)
                    psums[b] = psump.tile([2, 2 * F], FP32, tag="psum_row", name=f"psum_row_{b}")

                wsum_cols = wsums[b]
                psum_row = psums[b]
                if True:
                    # node(p, j) = n0 + p*jj + j: each partition reads
                    # jj KiB contiguous -> fully linear HBM->SBUF DMA.
                    x_sb = xp.tile([P, JJ, F], FP32, tag="x_sb")
                    nc.sync.dma_start(
                        out=x_sb[:, 0:jj, :],
                        in_=x_d[b, n0 : n0 + P * jj, :].rearrange(
                            "(p j) f -> p j f", p=P
                        ),
                    )
                    if b == 0 and ci == 3:
                        nc.sync.dma_start(out=w1rep8, in_=w1rep_bc)
                    g1 = g1p.tile([P, JJ, F], FP32, tag="g1")
                    # DVE takes the first jdve node-groups of the multiply,
                    # idle GpSimd the rest (they run on different SBUF port
                    # windows than the 1-port reduces). Priming chunks
                    # (jj < JJ) run DVE-only for the shortest latency chain.
                    prime = jj < JJ or (b == 0 and ci == 3)
                    jdve = jj if prime else jj - 2
                    if prime:
                        # per-group 2D ops against w1rep: shortest-latency
                        # chain, and no dependency on the w1rep8 fill DMA
                        for j in range(jj):
                            nc.vector.tensor_mul(g1[:, j, :], x_sb[:, j, :], w1rep)
                    else:
                        nc.vector.tensor_mul(
                            g1[:, 0:jdve, :].rearrange("p j f -> p (j f)"),
                            x_sb[:, 0:jdve, :].rearrange("p j f -> p (j f)"),
                            w1rep8[:, 0:jdve, :].rearrange("p j f -> p (j f)"),
                        )
                    if jdve < jj:
                        nc.gpsimd.tensor_mul(
                            g1[:, jdve:jj, :].rearrange("p j f -> p (j f)"),
                            x_sb[:, jdve:jj, :].rearrange("p j f -> p (j f)"),
                            w1rep8[:, jdve:jj, :].rearrange("p j f -> p (j f)"),
                        )
                    # Grouped 1x reduce: 5 groups on DVE, rest on ACT's
                    # fused accumulator (DVE marginal ~267ns/group, ACT
                    # ~790ns/group flat-heavy).
                    kdve = min(5, jj)
                    gates = small.tile([P, JJ], FP32, tag="gates")
                    nc.vector.reduce_sum(
                        gates[:, 0:kdve], g1[:, 0:kdve, :], axis=mybir.AxisListType.X
                    )
                    for j in range(kdve, jj):
                        scr2 = scrp.tile([P, F], FP32, tag="scr2")
                        nc.scalar.activation(
                            out=scr2,
                            in_=g1[:, j, :],
                            func=mybir.ActivationFunctionType.Copy,
                            bias=0.0,
                            scale=1.0,
                            accum_out=gates[:, j : j + 1],
                        )
                    w_sb = small.tile([P, JJ], FP32, tag="w")
                    nc.scalar.activation(
                        out=w_sb[:, 0:jj],
                        in_=gates[:, 0:jj],
                        func=mybir.ActivationFunctionType.Exp,
                        bias=b1b,
                        scale=1.0,
                        accum_out=wsum_cols[:, ci : ci + 1],
                    )
                    # Pair two weight columns per matmul: [128,2]
                    # stationary x N=512 moving. Row 0 cols 0:256 and row 1
                    # cols 256:512 hold the two wanted products; the host
                    # adds the halves. Halves PE instruction/sem-wait count.
                    for t in range(jj // 2):
                        nc.tensor.matmul(
                            out=psum_row,
                            lhsT=w_sb[:, 2 * t : 2 * t + 2],
                            rhs=x_sb[:, 2 * t : 2 * t + 2, :].rearrange(
                                "p j f -> p (j f)"
                            ),
                            start=(ci == 0 and t == 0),
                            stop=(ci == nchunks - 1 and t == jj // 2 - 1),
                        )
                if ci == nchunks - 1:
                    epilogue(b)


    if fixup:
        # CoreSim chokes on the inserted NoOps; only needed for the HW compile.
        split_multiwait(nc)
    return nc


def run(x, W1, b1, trace: bool = False, tmpdir: str | None = None):
    """Shard over cores, execute, and return (y, BassKernelResults)."""
    x = np.ascontiguousarray(np.asarray(x, dtype=np.float32))
    W1 = np.ascontiguousarray(np.asarray(W1, dtype=np.float32))
    b1 = np.ascontiguousarray(np.asarray(b1, dtype=np.float32))
    assert x.shape == (BZ, N, F), x.shape

    nc = build()
    in_maps = [
        {"x": np.ascontiguousarray(x[c * BZL : (c + 1) * BZL]), "W1": W1, "b1": b1}
        for c in range(NCORES)
    ]
    res = run_bass_kernel_spmd(
        nc, in_maps, core_ids=list(range(NCORES)), trace=trace, tmpdir=tmpdir
    )
    y2 = np.concatenate([r["y_unnorm"] for r in res.results], axis=0)  # [32,2,512]
    y_un = y2[:, 0, 0:F] + y2[:, 1, F : 2 * F]                           # [32, 256]
    ws = np.concatenate([r["wsum"] for r in res.results], axis=0)        # [32, 128, ns]
    denom = ws.reshape(BZ, -1).astype(np.float64).sum(axis=1)
    y = (y_un.astype(np.float64) / denom[:, None]).astype(np.float32)
    return y, res


def kernel(x, W1, b1):
    y, _ = run(x, W1, b1)
    return y



# revision 6
# speedup vs baseline: 1.0566x; 1.0566x over previous
"""Trainium2 Bass kernel for nn_Aggregate (gate-softmax graph pooling).

Computes, for each graph b:
    gate[b,n] = x[b,n,:] @ W1 + b1
    attn      = softmax(gate[b,:])
    y[b,:]    = sum_n attn[b,n] * x[b,n,:]

Strategy (memory-bound; roofline = one HBM read of x at ~360-384 GB/s/core
across 16 DMA engines => ~90us/core for 32 MiB):
  - Data-parallel over the 32 graphs: 4 graphs per NeuronCore, 8 cores.
  - Single pass over x; gate values are ~N(0,1) so exp() without the
    max-shift is safe in fp32.
  - Per 1 MiB slab (1024 nodes as [128 partitions x 8 nodes x 256 feat]):
      DVE    : 5 node-groups of fused mul+reduce (tensor_tensor_reduce:
               gates[:,j] = sum_f x[:,j,:]*W1rep) ~1.8us
      GpSimd : the other 3 groups via scalar_tensor_tensor accum ~1.5us
      ACT    : exp(gates + b1) with accum_out giving sum(e^g)/partition
      PE     : 4 matmuls in float32r (1 cycle/row for moving>=256, 4x
               faster than plain fp32): psum[2,512] += w_pair^T @ x_pair
               pairs-of-2 diagonal trick; host adds the two halves
  - All engines fit under the 2.73us/slab DMA window -> DMA-bound.
  - Epilogue (psum->SBUF copy + output DMAs) rides the ACT queue so the
    sync-queue x-load stream never stalls on compute.
  - Denominator finishes on host: sum of the per-partition exp-sums.
"""

import sys
import types

import numpy as np

import concourse.bass as bass
import concourse.tile as tile
from concourse import mybir
from concourse.bass_utils import run_bass_kernel_spmd

# bass_utils' axon trace path does `from antenv.axon_hooks import ...`, which
# this image doesn't ship; stub it so BASS_TRACE=1 degrades to a warning
# instead of an ImportError. (Our own profiling wires a real hook in test.py.)
if "antenv.axon_hooks" not in sys.modules:
    try:
        import antenv  # noqa: F401
        import antenv.axon_hooks  # noqa: F401
    except ImportError:
        _m = types.ModuleType("antenv.axon_hooks")
        _m._hook = None
        _m.set_axon_ntff_profile_hook = lambda h: setattr(_m, "_hook", h)
        _m.get_axon_ntff_profile_hook = lambda: _m._hook
        sys.modules["antenv.axon_hooks"] = _m

BZ, N, F = 32, 8192, 256
NCORES = 8
BZL = BZ // NCORES  # graphs per core
P = 128             # SBUF partitions
JJ = 8              # nodes per partition per slab
SLAB = P * JJ       # 1024 nodes per slab
FP32 = mybir.dt.float32
FP32R = mybir.dt.float32r
NDVE = 5            # node-groups of the gate reduce on DVE (rest on GpSimd)


def split_multiwait(nc) -> int:
    """Walrus in this image only encodes one sync-wait per instruction for
    ctrl-class ops; hoist extra waits onto single-wait NoOps just before."""
    n_fixed = 0
    for fn in nc.m.functions:
        for blk in fn.blocks:
            new_list = []
            for inst in blk.instructions:
                si = inst.sync_info
                waits = list(si.on_wait) if si is not None else []
                if len(waits) > 1:
                    for k, w in enumerate(waits):
                        new_list.append(
                            mybir.InstNoOp(
                                name=f"{inst.name}-wsplit{k}",
                                engine=inst.engine,
                                sync_info=mybir.SyncInfo(on_wait=[w], on_update=[]),
                                bass_nofuse=True,
                            )
                        )
                    inst.sync_info = mybir.SyncInfo(
                        on_wait=[], on_update=list(si.on_update)
                    )
                    n_fixed += 1
                new_list.append(inst)
            blk.instructions = new_list
    return n_fixed


def build(n_nodes: int = N, bzl: int = BZL, fixup: bool = True) -> bass.Bass:
    nslab = n_nodes // SLAB
    assert nslab * SLAB == n_nodes

    nc = bass.Bass("TRN2", target_bir_lowering=False, debug=False)
    x_d = nc.dram_tensor("x", [bzl, n_nodes, F], FP32, kind="ExternalInput").ap()
    w1_d = nc.dram_tensor("W1", [F, 1], FP32, kind="ExternalInput").ap()
    b1_d = nc.dram_tensor("b1", [1], FP32, kind="ExternalInput").ap()
    y_d = nc.dram_tensor("y_unnorm", [bzl, 2, 2 * F], FP32, kind="ExternalOutput").ap()
    ws_d = nc.dram_tensor("wsum", [bzl, P, nslab], FP32, kind="ExternalOutput").ap()

    with tile.TileContext(nc) as tc:
        with (
            tc.tile_pool(name="singles", bufs=1) as singles,
            tc.tile_pool(name="xp", bufs=12) as xp,
            tc.tile_pool(name="scrd", bufs=2) as scrd,
            tc.tile_pool(name="scrg", bufs=2) as scrg,
            tc.tile_pool(name="small", bufs=4) as small,
            tc.tile_pool(name="wsump", bufs=3) as wsump,
            tc.tile_pool(name="outp", bufs=2) as outp,
            tc.tile_pool(name="psum", bufs=2, space="PSUM") as psump,
        ):
            # b1 scalar broadcast to [128,1] (the exp bias) — first in the
            # sync queue so the first slab's exp is never blocked on it.
            b1b = singles.tile([P, 1], FP32)
            nc.sync.dma_start(
                out=b1b,
                in_=bass.AP(tensor=b1_d.tensor, offset=b1_d.offset, ap=[[0, P], [1, 1]]),
            )
            # W1 (256 contiguous fp32) broadcast to [128, 256].
            w1rep = singles.tile([P, F], FP32)
            nc.sync.dma_start(
                out=w1rep,
                in_=bass.AP(tensor=w1_d.tensor, offset=w1_d.offset, ap=[[0, P], [1, F]]),
            )
            # Materialized [128, JJ-NDVE, 256] copy of W1 for the GpSimd flat
            # multiply (unit-stride operands keep it on the fast path).
            # Filled by an SBUF->SBUF broadcast DMA right after w1rep lands.
            w1r_ap = w1rep[:, :]
            w1g = singles.tile([P, JJ - NDVE, F], FP32)
            nc.sync.dma_start(
                out=w1g,
                in_=bass.AP(
                    tensor=w1r_ap.tensor,
                    offset=w1r_ap.offset,
                    ap=[list(w1r_ap.ap[0]), [0, JJ - NDVE], list(w1r_ap.ap[1])],
                ),
            )
            # Dummy exp so ACT's table set loads during the preamble instead
            # of on the first real exp.
            warm = singles.tile([P, 1], FP32)
            nc.scalar.activation(
                out=warm, in_=b1b, func=mybir.ActivationFunctionType.Exp,
                bias=0.0, scale=1.0,
            )

            wsums = {}
            psums = {}

            for b in range(bzl):
                for s in range(nslab):
                    if s == 0:
                        wsums[b] = wsump.tile([P, nslab], FP32, tag="wsum", name=f"wsum_{b}")
                        psums[b] = psump.tile([2, 2 * F], FP32, tag="psum_row", name=f"psum_row_{b}")
                    wsum_cols = wsums[b]
                    psum_row = psums[b]

                    # node(p, j) = s*SLAB + p*JJ + j: each partition reads
                    # 8 KiB contiguous -> fully linear HBM->SBUF DMA.
                    # Tile dtype is float32r (same bits as fp32) so the BIR
                    # verifier accepts it as an FP32r matmult operand; the
                    # vector engines read it through a plain-fp32 bitcast.
                    x_sb = xp.tile([P, JJ, F], FP32R, tag="x_sb")
                    nc.sync.dma_start(
                        out=x_sb,
                        in_=x_d[b, s * SLAB : (s + 1) * SLAB, :].rearrange(
                            "(p j) f -> p j f", p=P
                        ).bitcast(FP32R),
                    )
                    x_f32 = x_sb[:, :, :].bitcast(FP32)
                    gates = small.tile([P, JJ], FP32, tag="gates")
                    # Gate compute, split three ways so every engine stays
                    # under the DMA window:
                    #  - DVE: NDVE node-groups of fused multiply+reduce
                    #    (scalar_tensor_tensor: out=(x*1)*W1, accum=gate).
                    #  - GpSimd: one flat multiply for the remaining groups.
                    #  - ACT: accum-copy reduce of those groups (+ the exp).
                    for j in range(NDVE):
                        g1 = scrd.tile([P, F], FP32, tag="g1d")
                        nc.vector.scalar_tensor_tensor(
                            out=g1,
                            in0=x_f32[:, j, :],
                            scalar=1.0,
                            in1=w1rep,
                            op0=mybir.AluOpType.mult,
                            op1=mybir.AluOpType.mult,
                            accum_out=gates[:, j : j + 1],
                        )
                    ngp = JJ - NDVE
                    g2 = scrg.tile([P, ngp, F], FP32, tag="g1g")
                    nc.gpsimd.tensor_mul(
                        g2[:, :, :].rearrange("p j f -> p (j f)"),
                        x_f32[:, NDVE:JJ, :].rearrange("p j f -> p (j f)"),
                        w1g,
                    )
                    for jj in range(ngp):
                        dump = scrd.tile([P, F], FP32, tag="dump")
                        nc.scalar.activation(
                            out=dump,
                            in_=g2[:, jj, :],
                            func=mybir.ActivationFunctionType.Copy,
                            bias=0.0,
                            scale=1.0,
                            accum_out=gates[:, NDVE + jj : NDVE + jj + 1],
                        )
                    w_sb = small.tile([P, JJ], FP32R, tag="w")
                    nc.scalar.activation(
                        out=w_sb,
                        in_=gates,
                        func=mybir.ActivationFunctionType.Exp,
                        bias=b1b,
                        scale=1.0,
                        accum_out=wsum_cols[:, s : s + 1],
                    )
                    # Pair two weight columns per matmul: [128,2] stationary
                    # x N=512 moving, in float32r (1 cycle/row for moving
                    # >=256 vs 4 for plain fp32). Row 0 cols 0:256 and row 1
                    # cols 256:512 hold the two wanted products; the host
                    # adds the halves.
                    for t in range(JJ // 2):
                        nc.tensor.matmul(
                            out=psum_row,
                            lhsT=w_sb[:, 2 * t : 2 * t + 2],
                            rhs=x_sb[:, 2 * t : 2 * t + 2, :].rearrange(
                                "p j f -> p (j f)"
                            ),
                            start=(s == 0 and t == 0),
                            stop=(s == nslab - 1 and t == JJ // 2 - 1),
                        )
                    if s == nslab - 1:
                        # Epilogue rides the ACT queue: psum->SBUF copy, then
                        # output DMAs issued by ACT so the sync queue's x-load
                        # stream never waits on compute.
                        yrow = outp.tile([2, 2 * F], FP32)
                        nc.scalar.copy(yrow, psum_row)
                        nc.scalar.dma_start(out=y_d[b], in_=yrow)
                        nc.scalar.dma_start(out=ws_d[b], in_=wsum_cols)

    if fixup:
        # CoreSim chokes on the inserted NoOps; only needed for the HW compile.
        split_multiwait(nc)
    return nc


def run(x, W1, b1, trace: bool = False, tmpdir: str | None = None):
    """Shard over cores, execute, and return (y, BassKernelResults)."""
    x = np.ascontiguousarray(np.asarray(x, dtype=np.float32))
    W1 = np.ascontiguousarray(np.asarray(W1, dtype=np.float32))
    b1 = np.ascontiguousarray(np.asarray(b1, dtype=np.float32))
    assert x.shape == (BZ, N, F), x.shape

    nc = build()
    in_maps = [
        {"x": np.ascontiguousarray(x[c * BZL : (c + 1) * BZL]), "W1": W1, "b1": b1}
        for c in range(NCORES)
    ]
    res = run_bass_kernel_spmd(
        nc, in_maps, core_ids=list(range(NCORES)), trace=trace, tmpdir=tmpdir
    )
    y2 = np.concatenate([r["y_unnorm"] for r in res.results], axis=0)  # [32,2,512]
    y_un = y2[:, 0, 0:F] + y2[:, 1, F : 2 * F]                           # [32, 256]
    ws = np.concatenate([r["wsum"] for r in res.results], axis=0)        # [32, 128, ns]
    denom = ws.reshape(BZ, -1).astype(np.float64).sum(axis=1)
    y = (y_un.astype(np.float64) / denom[:, None]).astype(np.float32)
    return y, res


def kernel(x, W1, b1):
    y, _ = run(x, W1, b1)
    return y
